# revision 19
# baseline (speedup 1.0000x reference)
"""Trainium2 Bass kernel for nn_AdaptiveTensorUnit.

Strategy (data-parallel over the unit axis N, per the sharding hint):
 - Shard the N=262144 units across 8 NeuronCores (32768 each).
 - Host-side prep (sharding/layout): compute integer cells, gather each
   unit's 128 local field samples (pure data staging), pre-transpose all
   per-unit tensors to feature-major [128, NS] layout so the device matmuls
   contract over partitions. Bias b2 is folded into the shipped signatures
   (sig* = sig - b2), standard constant folding.
 - Device (per core), per 512-unit tile and for both position evaluations:
     pre  = W1top.T @ sig*+b2(bf16) + W1bot.T @ local(bf16)      [PE]
     h    = tanh(pre + b1)                                        [ACT]
     dps  = W2.T @ h - sig*   (via -I stationary)                 [PE]
     sq   = dps * dps -> bf16                                     [DVE]
     d2   = sum over partitions (GPSIMD partition_all_reduce)
     row0 of the broadcast d2 -> DRAM scratch                     [DMA]
   Epilogue: reload d2 as [128, 256] columns, accept = d2n <= d2o,
   stability = sqrt(min), final_pos = pos + accept*offset, DMA out.
"""

import numpy as np
import ml_dtypes

N = 262144
G = 128
D = 128
NCORES = 8
NS = N // NCORES          # 32768 units per core
T = 512                   # units per matmul tile (moving dim)
NT = NS // T              # 64 tiles
UC = NS // 128            # 256 columns of 128 units

# Neighborhood offsets: first 128 of the 7x7x7 cube in i-major (i,j,k) order.
_grid = np.stack(
    np.meshgrid(np.arange(-3, 4), np.arange(-3, 4), np.arange(-3, 4), indexing="ij"),
    -1,
).reshape(-1, 3)[:D]
_OI = _grid[:, 0].astype(np.int32)
_OJ = _grid[:, 1].astype(np.int32)
_OK = _grid[:, 2].astype(np.int32)

_GRAPH = None


def _build_graph():
    import concourse.bass as bass
    import concourse.mybir as mybir
    import concourse.tile as tile
    import concourse.bass_isa as bass_isa
    from concourse import bacc

    f32 = mybir.dt.float32
    bf16 = mybir.dt.bfloat16
    AF = mybir.ActivationFunctionType
    ALU = mybir.AluOpType

    nc = bacc.Bacc(None, target_bir_lowering=False, debug=False)

    sigbT = nc.dram_tensor("sigbT", [128, NS], bf16, kind="ExternalInput")
    locoldT = nc.dram_tensor("locoldT", [128, NS], bf16, kind="ExternalInput")
    locnewT = nc.dram_tensor("locnewT", [128, NS], bf16, kind="ExternalInput")
    w1t_d = nc.dram_tensor("W1top", [128, 128], bf16, kind="ExternalInput")
    w1b_d = nc.dram_tensor("W1bot", [128, 128], bf16, kind="ExternalInput")
    w2_d = nc.dram_tensor("W2c", [128, 128], bf16, kind="ExternalInput")
    negI_d = nc.dram_tensor("negI", [128, 128], bf16, kind="ExternalInput")
    onesm_d = nc.dram_tensor("onesm", [128, 128], bf16, kind="ExternalInput")
    b1_d = nc.dram_tensor("b1c", [128, 1], f32, kind="ExternalInput")
    posu_d = nc.dram_tensor("posu", [128, UC, 3], f32, kind="ExternalInput")
    offu_d = nc.dram_tensor("offu", [128, UC, 3], f32, kind="ExternalInput")
    stab_d = nc.dram_tensor("stab", [128, UC], f32, kind="ExternalOutput")
    fpos_d = nc.dram_tensor("fpos", [128, UC, 3], f32, kind="ExternalOutput")
    d2o_d = nc.dram_tensor("d2o_scratch", [128, UC], f32)
    d2n_d = nc.dram_tensor("d2n_scratch", [128, UC], f32)

    with tile.TileContext(nc) as tc:
        with (
            tc.tile_pool(name="singles", bufs=1) as singles,
            tc.tile_pool(name="persist", bufs=1) as persist,
            tc.tile_pool(name="stream", bufs=4) as stream,
            tc.tile_pool(name="work", bufs=4) as work,
            tc.tile_pool(name="psum_mm", bufs=3, space="PSUM") as psum_mm,
            tc.tile_pool(name="psum_d2", bufs=1, space="PSUM") as psum_d2,
        ):
            w1t = singles.tile([128, 128], bf16, tag="w1t")
            w1b = singles.tile([128, 128], bf16, tag="w1b")
            w2 = singles.tile([128, 128], bf16, tag="w2")
            negI = singles.tile([128, 128], bf16, tag="negI")
            onesm = singles.tile([128, 128], bf16, tag="onesm")
            b1 = singles.tile([128, 1], f32, tag="b1")
            posu = singles.tile([128, UC, 3], f32, tag="posu")
            offu = singles.tile([128, UC, 3], f32, tag="offu")
            nc.sync.dma_start(out=w1t[:], in_=w1t_d[:])
            nc.sync.dma_start(out=w1b[:], in_=w1b_d[:])
            nc.sync.dma_start(out=w2[:], in_=w2_d[:])
            nc.sync.dma_start(out=negI[:], in_=negI_d[:])
            nc.sync.dma_start(out=onesm[:], in_=onesm_d[:])
            nc.sync.dma_start(out=b1[:], in_=b1_d[:])
            nc.sync.dma_start(out=posu[:], in_=posu_d[:])
            nc.sync.dma_start(out=offu[:], in_=offu_d[:])

            # d2 staging rows: partition r holds units [r*8192, (r+1)*8192).
            d2o_st = persist.tile([128, NS // 4], f32, tag="d2ost")
            d2n_st = persist.tile([128, NS // 4], f32, tag="d2nst")

            for t in range(NT):
                us = t * T
                sigb_t = stream.tile([128, T], bf16, tag="sigb")
                nc.sync.dma_start(out=sigb_t[:], in_=sigbT[:, us : us + T])
                loco_t = stream.tile([128, T], bf16, tag="loco")
                nc.scalar.dma_start(out=loco_t[:], in_=locoldT[:, us : us + T])
                locn_t = stream.tile([128, T], bf16, tag="locn")
                nc.scalar.dma_start(out=locn_t[:], in_=locnewT[:, us : us + T])

                # Both evals side by side in [128, 1024] PSUM tiles
                # (cols 0:512 = old, 512:1024 = new); matmuls stay 512-wide,
                # ACT ops run once per pair.
                pre2 = psum_mm.tile([128, 2 * T], f32, tag="mm")
                nc.tensor.matmul(pre2[:, 0:T], lhsT=w1t[:], rhs=sigb_t[:], start=True, stop=False)
                nc.tensor.matmul(pre2[:, T : 2 * T], lhsT=w1t[:], rhs=sigb_t[:], start=True, stop=False)
                nc.tensor.matmul(pre2[:, 0:T], lhsT=w1b[:], rhs=loco_t[:], start=False, stop=True)
                nc.tensor.matmul(pre2[:, T : 2 * T], lhsT=w1b[:], rhs=locn_t[:], start=False, stop=True)

                h2 = work.tile([128, 2 * T], bf16, tag="h")
                nc.scalar.activation(h2[:], pre2[:], AF.Tanh, bias=b1[:])

                dps2 = psum_mm.tile([128, 2 * T], f32, tag="mm")
                nc.tensor.matmul(dps2[:, 0:T], lhsT=w2[:], rhs=h2[:, 0:T], start=True, stop=False)
                nc.tensor.matmul(dps2[:, T : 2 * T], lhsT=w2[:], rhs=h2[:, T : 2 * T], start=True, stop=False)
                nc.tensor.matmul(dps2[:, 0:T], lhsT=negI[:], rhs=sigb_t[:], start=False, stop=True)
                nc.tensor.matmul(dps2[:, T : 2 * T], lhsT=negI[:], rhs=sigb_t[:], start=False, stop=True)

                sq2 = work.tile([128, 2 * T], bf16, tag="sq")
                nc.scalar.activation(sq2[:], dps2[:], AF.Square)

                d2p2 = psum_d2.tile([128, 2 * T], f32, tag="d2")
                nc.tensor.matmul(d2p2[:, 0:T], lhsT=onesm[:], rhs=sq2[:, 0:T], start=True, stop=True)
                nc.tensor.matmul(d2p2[:, T : 2 * T], lhsT=onesm[:], rhs=sq2[:, T : 2 * T], start=True, stop=True)

                r, col = 32 * (t // 16), (t % 16) * T
                nc.vector.tensor_copy(
                    d2o_st[r : r + 1, col : col + T], d2p2[r : r + 1, 0:T]
                )
                nc.vector.tensor_copy(
                    d2n_st[r : r + 1, col : col + T], d2p2[r : r + 1, T : 2 * T]
                )

            def _rows4(tile_ap):
                ap = tile_ap[:]
                return bass.AP(
                    tensor=ap.tensor,
                    offset=ap.offset,
                    ap=[[ap.ap[0][0] * 32, 4]] + list(ap.ap[1:]),
                )

            d2o_rows = _rows4(d2o_st)
            d2n_rows = _rows4(d2n_st)
            nc.sync.dma_start(out=d2o_d[:], in_=d2o_rows)
            nc.sync.dma_start(out=d2n_d[:], in_=d2n_rows)

            # Epilogue: accept, stability, final positions.
            d2o_sb = persist.tile([128, UC], f32, tag="d2o")
            d2n_sb = persist.tile([128, UC], f32, tag="d2n")
            acc_sb = persist.tile([128, UC], f32, tag="acc")
            min_sb = persist.tile([128, UC], f32, tag="min")
            stab_sb = persist.tile([128, UC], f32, tag="stab")
            fp_sb = persist.tile([128, UC, 3], f32, tag="fp")
            nc.sync.dma_start(out=d2o_sb[:], in_=d2o_d[:])
            nc.sync.dma_start(out=d2n_sb[:], in_=d2n_d[:])
            nc.vector.tensor_tensor(
                out=acc_sb[:], in0=d2n_sb[:], in1=d2o_sb[:], op=ALU.is_le
            )
            nc.vector.tensor_tensor(
                out=min_sb[:], in0=d2n_sb[:], in1=d2o_sb[:], op=ALU.min
            )
            nc.scalar.activation(stab_sb[:], min_sb[:], AF.Sqrt)
            nc.sync.dma_start(out=stab_d[:], in_=stab_sb[:])
            for c in range(3):
                nc.vector.tensor_tensor(
                    out=fp_sb[:, :, c], in0=offu[:, :, c], in1=acc_sb[:], op=ALU.mult
                )
                nc.vector.tensor_tensor(
                    out=fp_sb[:, :, c], in0=fp_sb[:, :, c], in1=posu[:, :, c], op=ALU.add
                )
            nc.sync.dma_start(out=fpos_d[:], in_=fp_sb[:])

    nc.finalize()
    return nc


def get_graph():
    global _GRAPH
    if _GRAPH is None:
        _GRAPH = _build_graph()
    return _GRAPH


def _locals_of(field, pos):
    pc = np.clip(pos.astype(np.int32), 0, G - 1)
    xx = np.clip(pc[:, 0:1] + _OI[None, :], 0, G - 1)
    yy = np.clip(pc[:, 1:2] + _OJ[None, :], 0, G - 1)
    zz = np.clip(pc[:, 2:3] + _OK[None, :], 0, G - 1)
    return field[xx, yy, zz]


def prepare_in_maps(universe_field, positions, signatures, offsets, W1, b1, W2, b2):
    field = np.asarray(universe_field, np.float32)
    pos = np.asarray(positions, np.float32)
    sig = np.asarray(signatures, np.float32)
    off = np.asarray(offsets, np.float32)
    W1 = np.asarray(W1, np.float32)
    b1 = np.asarray(b1, np.float32)
    W2 = np.asarray(W2, np.float32)
    b2 = np.asarray(b2, np.float32)
    test_pos = pos + off

    lo = _locals_of(field, pos)
    ln = _locals_of(field, test_pos)
    sigstar = sig - b2[None, :]

    bf = ml_dtypes.bfloat16
    w1top = np.ascontiguousarray(W1[:D]).astype(bf)
    w1bot = np.ascontiguousarray(W1[D:]).astype(bf)
    w2c = np.ascontiguousarray(W2).astype(bf)
    negI = (-np.eye(128, dtype=np.float32)).astype(bf)
    onesm = np.ones((128, 128), np.float32).astype(bf)
    b1c = b1.reshape(128, 1).astype(np.float32)

    in_maps = []
    for c in range(NCORES):
        sl = slice(c * NS, (c + 1) * NS)
        in_maps.append(
            {
                "sigbT": np.ascontiguousarray(sigstar[sl].T).astype(bf),
                "locoldT": np.ascontiguousarray(lo[sl].T).astype(bf),
                "locnewT": np.ascontiguousarray(ln[sl].T).astype(bf),
                "W1top": w1top,
                "W1bot": w1bot,
                "W2c": w2c,
                "negI": negI,
                "onesm": onesm,
                "b1c": b1c,
                "posu": np.ascontiguousarray(pos[sl].reshape(128, UC, 3)),
                "offu": np.ascontiguousarray(off[sl].reshape(128, UC, 3)),
            }
        )
    return in_maps


def collect_outputs(results):
    stab = np.empty(N, np.float32)
    fp = np.empty((N, 3), np.float32)
    for c in range(NCORES):
        r = results[c]
        stab[c * NS : (c + 1) * NS] = np.asarray(r["stab"]).reshape(NS)
        fp[c * NS : (c + 1) * NS] = np.asarray(r["fpos"]).reshape(NS, 3)
    return stab, fp


def kernel(universe_field, positions, signatures, offsets, W1, b1, W2, b2):
    from concourse.bass_utils import run_bass_kernel_spmd

    in_maps = prepare_in_maps(
        universe_field, positions, signatures, offsets, W1, b1, W2, b2
    )
    nc = get_graph()
    res = run_bass_kernel_spmd(nc, in_maps, core_ids=list(range(NCORES)))
    return collect_outputs(res.results)


# revision 20
# speedup vs baseline: 1.2156x; 1.2156x over previous
"""Trainium2 Bass kernel for nn_AdaptiveTensorUnit.

Strategy (data-parallel over the unit axis N, per the sharding hint):
 - Shard the N=262144 units across 8 NeuronCores (32768 each).
 - Host-side prep (sharding/layout): compute integer cells, gather each
   unit's 128 local field samples (pure data staging), pre-transpose all
   per-unit tensors to feature-major [128, NS] layout so the device matmuls
   contract over partitions. Bias b2 is folded into the shipped signatures
   (sig* = sig - b2), standard constant folding.
 - Device (per core), per 512-unit tile and for both position evaluations:
     pre  = W1top.T @ sig*+b2(bf16) + W1bot.T @ local(bf16)      [PE]
     h    = tanh(pre + b1)                                        [ACT]
     dps  = W2.T @ h - sig*   (via -I stationary)                 [PE]
     sq   = dps * dps -> bf16                                     [DVE]
     d2   = sum over partitions (GPSIMD partition_all_reduce)
     row0 of the broadcast d2 -> DRAM scratch                     [DMA]
   Epilogue: reload d2 as [128, 256] columns, accept = d2n <= d2o,
   stability = sqrt(min), final_pos = pos + accept*offset, DMA out.
"""

import numpy as np
import ml_dtypes

N = 262144
G = 128
D = 128
NCORES = 8
NS = N // NCORES          # 32768 units per core
T = 512                   # units per matmul tile (moving dim)
NT = NS // T              # 64 tiles
UC = NS // 128            # 256 columns of 128 units

# Neighborhood offsets: first 128 of the 7x7x7 cube in i-major (i,j,k) order.
_grid = np.stack(
    np.meshgrid(np.arange(-3, 4), np.arange(-3, 4), np.arange(-3, 4), indexing="ij"),
    -1,
).reshape(-1, 3)[:D]
_OI = _grid[:, 0].astype(np.int32)
_OJ = _grid[:, 1].astype(np.int32)
_OK = _grid[:, 2].astype(np.int32)

_GRAPH = None


def _build_graph():
    import concourse.bass as bass
    import concourse.mybir as mybir
    import concourse.tile as tile
    import concourse.bass_isa as bass_isa
    from concourse import bacc

    f32 = mybir.dt.float32
    bf16 = mybir.dt.bfloat16
    AF = mybir.ActivationFunctionType
    ALU = mybir.AluOpType

    nc = bacc.Bacc(None, target_bir_lowering=False, debug=False)

    sigbT = nc.dram_tensor("sigbT", [128, NS], bf16, kind="ExternalInput")
    locoldT = nc.dram_tensor("locoldT", [128, NS], bf16, kind="ExternalInput")
    locnewT = nc.dram_tensor("locnewT", [128, NS], bf16, kind="ExternalInput")
    w1t_d = nc.dram_tensor("W1top", [128, 128], bf16, kind="ExternalInput")
    w1b_d = nc.dram_tensor("W1bot", [128, 128], bf16, kind="ExternalInput")
    w2_d = nc.dram_tensor("W2c", [128, 128], bf16, kind="ExternalInput")
    negI_d = nc.dram_tensor("negI", [128, 128], bf16, kind="ExternalInput")
    onesm_d = nc.dram_tensor("onesm", [128, 128], bf16, kind="ExternalInput")
    b1_d = nc.dram_tensor("b1c", [128, 1], f32, kind="ExternalInput")
    posu_d = nc.dram_tensor("posu", [128, UC, 3], f32, kind="ExternalInput")
    offu_d = nc.dram_tensor("offu", [128, UC, 3], f32, kind="ExternalInput")
    stab_d = nc.dram_tensor("stab", [128, UC], f32, kind="ExternalOutput")
    fpos_d = nc.dram_tensor("fpos", [128, UC, 3], f32, kind="ExternalOutput")
    d2o_d = nc.dram_tensor("d2o_scratch", [128, UC], f32)
    d2n_d = nc.dram_tensor("d2n_scratch", [128, UC], f32)

    with tile.TileContext(nc) as tc:
        with (
            tc.tile_pool(name="singles", bufs=1) as singles,
            tc.tile_pool(name="persist", bufs=1) as persist,
            tc.tile_pool(name="stream", bufs=6) as stream,
            tc.tile_pool(name="work", bufs=6) as work,
            tc.tile_pool(name="psum_mm", bufs=6, space="PSUM") as psum_mm,
            tc.tile_pool(name="psum_d2", bufs=2, space="PSUM") as psum_d2,
        ):
            w1t = singles.tile([128, 128], bf16, tag="w1t")
            w1b = singles.tile([128, 128], bf16, tag="w1b")
            w2 = singles.tile([128, 128], bf16, tag="w2")
            negI = singles.tile([128, 128], bf16, tag="negI")
            onesm = singles.tile([128, 128], bf16, tag="onesm")
            b1 = singles.tile([128, 1], f32, tag="b1")
            posu = singles.tile([128, UC, 3], f32, tag="posu")
            offu = singles.tile([128, UC, 3], f32, tag="offu")
            nc.sync.dma_start(out=w1t[:], in_=w1t_d[:])
            nc.sync.dma_start(out=w1b[:], in_=w1b_d[:])
            nc.sync.dma_start(out=w2[:], in_=w2_d[:])
            nc.sync.dma_start(out=negI[:], in_=negI_d[:])
            nc.sync.dma_start(out=onesm[:], in_=onesm_d[:])
            nc.sync.dma_start(out=b1[:], in_=b1_d[:])
            nc.sync.dma_start(out=posu[:], in_=posu_d[:])
            nc.sync.dma_start(out=offu[:], in_=offu_d[:])

            # d2 staging rows: partition r holds units [r*8192, (r+1)*8192).
            d2o_st = persist.tile([128, NS // 4], f32, tag="d2ost")
            d2n_st = persist.tile([128, NS // 4], f32, tag="d2nst")

            for t in range(NT):
                us = t * T
                sigb_t = stream.tile([128, T], bf16, tag="sigb")
                nc.sync.dma_start(out=sigb_t[:], in_=sigbT[:, us : us + T])
                loco_t = stream.tile([128, T], bf16, tag="loco")
                nc.scalar.dma_start(out=loco_t[:], in_=locoldT[:, us : us + T])
                locn_t = stream.tile([128, T], bf16, tag="locn")
                nc.scalar.dma_start(out=locn_t[:], in_=locnewT[:, us : us + T])

                # Layer 1, stationaries shared across both evals.
                preO = psum_mm.tile([128, T], f32, tag="mm")
                preN = psum_mm.tile([128, T], f32, tag="mm")
                nc.tensor.matmul(preO[:], lhsT=w1t[:], rhs=sigb_t[:], start=True, stop=False)
                nc.tensor.matmul(preN[:], lhsT=w1t[:], rhs=sigb_t[:], start=True, stop=False)
                nc.tensor.matmul(preO[:], lhsT=w1b[:], rhs=loco_t[:], start=False, stop=True)
                nc.tensor.matmul(preN[:], lhsT=w1b[:], rhs=locn_t[:], start=False, stop=True)

                hO = work.tile([128, T], bf16, tag="h")
                hN = work.tile([128, T], bf16, tag="h")
                nc.scalar.activation(hO[:], preO[:], AF.Tanh, bias=b1[:])
                nc.scalar.activation(hN[:], preN[:], AF.Tanh, bias=b1[:])

                dpsO = psum_mm.tile([128, T], f32, tag="mm")
                dpsN = psum_mm.tile([128, T], f32, tag="mm")
                nc.tensor.matmul(dpsO[:], lhsT=w2[:], rhs=hO[:], start=True, stop=False)
                nc.tensor.matmul(dpsN[:], lhsT=w2[:], rhs=hN[:], start=True, stop=False)
                nc.tensor.matmul(dpsO[:], lhsT=negI[:], rhs=sigb_t[:], start=False, stop=True)
                nc.tensor.matmul(dpsN[:], lhsT=negI[:], rhs=sigb_t[:], start=False, stop=True)

                sqO = work.tile([128, T], bf16, tag="sq")
                sqN = work.tile([128, T], bf16, tag="sq")
                nc.scalar.activation(sqO[:], dpsO[:], AF.Square)
                nc.scalar.activation(sqN[:], dpsN[:], AF.Square)

                d2pO = psum_d2.tile([128, T], f32, tag="d2")
                d2pN = psum_d2.tile([128, T], f32, tag="d2")
                nc.tensor.matmul(d2pO[:], lhsT=onesm[:], rhs=sqO[:], start=True, stop=True)
                nc.tensor.matmul(d2pN[:], lhsT=onesm[:], rhs=sqN[:], start=True, stop=True)

                r, col = 32 * (t // 16), (t % 16) * T
                nc.vector.tensor_copy(
                    d2o_st[r : r + 1, col : col + T], d2pO[r : r + 1, :]
                )
                nc.vector.tensor_copy(
                    d2n_st[r : r + 1, col : col + T], d2pN[r : r + 1, :]
                )

            def _rows4(tile_ap):
                ap = tile_ap[:]
                return bass.AP(
                    tensor=ap.tensor,
                    offset=ap.offset,
                    ap=[[ap.ap[0][0] * 32, 4]] + list(ap.ap[1:]),
                )

            d2o_rows = _rows4(d2o_st)
            d2n_rows = _rows4(d2n_st)
            nc.sync.dma_start(out=d2o_d[:], in_=d2o_rows)
            nc.sync.dma_start(out=d2n_d[:], in_=d2n_rows)

            # Epilogue: accept, stability, final positions.
            d2o_sb = persist.tile([128, UC], f32, tag="d2o")
            d2n_sb = persist.tile([128, UC], f32, tag="d2n")
            acc_sb = persist.tile([128, UC], f32, tag="acc")
            min_sb = persist.tile([128, UC], f32, tag="min")
            stab_sb = persist.tile([128, UC], f32, tag="stab")
            fp_sb = persist.tile([128, UC, 3], f32, tag="fp")
            nc.sync.dma_start(out=d2o_sb[:], in_=d2o_d[:])
            nc.sync.dma_start(out=d2n_sb[:], in_=d2n_d[:])
            nc.vector.tensor_tensor(
                out=acc_sb[:], in0=d2n_sb[:], in1=d2o_sb[:], op=ALU.is_le
            )
            nc.vector.tensor_tensor(
                out=min_sb[:], in0=d2n_sb[:], in1=d2o_sb[:], op=ALU.min
            )
            nc.scalar.activation(stab_sb[:], min_sb[:], AF.Sqrt)
            nc.sync.dma_start(out=stab_d[:], in_=stab_sb[:])
            for c in range(3):
                nc.vector.tensor_tensor(
                    out=fp_sb[:, :, c], in0=offu[:, :, c], in1=acc_sb[:], op=ALU.mult
                )
                nc.vector.tensor_tensor(
                    out=fp_sb[:, :, c], in0=fp_sb[:, :, c], in1=posu[:, :, c], op=ALU.add
                )
            nc.sync.dma_start(out=fpos_d[:], in_=fp_sb[:])

    nc.finalize()
    return nc


def get_graph():
    global _GRAPH
    if _GRAPH is None:
        _GRAPH = _build_graph()
    return _GRAPH


def _locals_of(field, pos):
    pc = np.clip(pos.astype(np.int32), 0, G - 1)
    xx = np.clip(pc[:, 0:1] + _OI[None, :], 0, G - 1)
    yy = np.clip(pc[:, 1:2] + _OJ[None, :], 0, G - 1)
    zz = np.clip(pc[:, 2:3] + _OK[None, :], 0, G - 1)
    return field[xx, yy, zz]


def prepare_in_maps(universe_field, positions, signatures, offsets, W1, b1, W2, b2):
    field = np.asarray(universe_field, np.float32)
    pos = np.asarray(positions, np.float32)
    sig = np.asarray(signatures, np.float32)
    off = np.asarray(offsets, np.float32)
    W1 = np.asarray(W1, np.float32)
    b1 = np.asarray(b1, np.float32)
    W2 = np.asarray(W2, np.float32)
    b2 = np.asarray(b2, np.float32)
    test_pos = pos + off

    lo = _locals_of(field, pos)
    ln = _locals_of(field, test_pos)
    sigstar = sig - b2[None, :]

    bf = ml_dtypes.bfloat16
    w1top = np.ascontiguousarray(W1[:D]).astype(bf)
    w1bot = np.ascontiguousarray(W1[D:]).astype(bf)
    w2c = np.ascontiguousarray(W2).astype(bf)
    negI = (-np.eye(128, dtype=np.float32)).astype(bf)
    onesm = np.ones((128, 128), np.float32).astype(bf)
    b1c = b1.reshape(128, 1).astype(np.float32)

    in_maps = []
    for c in range(NCORES):
        sl = slice(c * NS, (c + 1) * NS)
        in_maps.append(
            {
                "sigbT": np.ascontiguousarray(sigstar[sl].T).astype(bf),
                "locoldT": np.ascontiguousarray(lo[sl].T).astype(bf),
                "locnewT": np.ascontiguousarray(ln[sl].T).astype(bf),
                "W1top": w1top,
                "W1bot": w1bot,
                "W2c": w2c,
                "negI": negI,
                "onesm": onesm,
                "b1c": b1c,
                "posu": np.ascontiguousarray(pos[sl].reshape(128, UC, 3)),
                "offu": np.ascontiguousarray(off[sl].reshape(128, UC, 3)),
            }
        )
    return in_maps


def collect_outputs(results):
    stab = np.empty(N, np.float32)
    fp = np.empty((N, 3), np.float32)
    for c in range(NCORES):
        r = results[c]
        stab[c * NS : (c + 1) * NS] = np.asarray(r["stab"]).reshape(NS)
        fp[c * NS : (c + 1) * NS] = np.asarray(r["fpos"]).reshape(NS, 3)
    return stab, fp


def kernel(universe_field, positions, signatures, offsets, W1, b1, W2, b2):
    from concourse.bass_utils import run_bass_kernel_spmd

    in_maps = prepare_in_maps(
        universe_field, positions, signatures, offsets, W1, b1, W2, b2
    )
    nc = get_graph()
    res = run_bass_kernel_spmd(nc, in_maps, core_ids=list(range(NCORES)))
    return collect_outputs(res.results)
